# revision 1
# baseline (speedup 1.0000x reference)
"""Trainium2 Bass kernel for a 3-layer LIF spiking network (T=128, B=32,
1024 -> 2048 -> 2048 -> 1024), data-parallel over batch on 8 NeuronCores.

Strategy per core (B_local = 4):
  - The time scan is only elementwise; the matmuls are batched over all
    T timesteps per layer: syn[l] = spikes[l-1] @ W[l].T + b[l] with free
    dim (t, b) = 512, K-accumulated in PSUM, evicted to SBUF with the
    per-partition bias added on the Scalar engine.
  - The LIF recurrence m' = beta*(m - (m>1)) + syn runs as ONE custom DVE
    instruction per (layer, timestep); the spike output s = (m > 1) is a
    tensor_scalar is_gt written directly into the next layer's matmul rhs
    layout via a strided access pattern.
  - Time is chunked (2 chunks of 64 steps) so scans overlap later matmuls.

All DRAM I/O is contiguous: weights are transposed and inputs/outputs are
laid out host-side in numpy.
"""

import numpy as np

import concourse.bacc as bacc
import concourse.mybir as mybir
import concourse.tile as tile
import concourse.dve_ops as dve_ops
from concourse.dve_ops import DveOp, OPS, _SUB_OPCODE_FOR_NAME
from concourse.dve_spec import Spec, Src0, Src1, C0, One, lower, _has_src1
from concourse.dve_uop import DveOpSpec
from concourse.bass_utils import run_bass_kernel_spmd

FP = mybir.dt.float32
FH = mybir.dt.float16
N_CORES = 8
T = 128
B = 32
BL = B // N_CORES          # 4 batch elements per core
TB = T * BL                # 512 = matmul free dim over all timesteps
SIZES = [1024, 2048, 2048, 1024]
BETA = 0.95
THRESH = 1.0
TC = 64                    # timesteps per chunk
NCH = T // TC              # 2 chunks
OCG = 4                    # output-neuron chunks (of 128) per PSUM group

# fp16x2 exact-weight split: W*2^12 = Whi + Wlo (both fp16, normal range);
# spikes are emitted as s*2^-12 so each product recovers s*w exactly.
WSCALE = 4096.0
SINV = 1.0 / WSCALE


def _register_lif_op() -> DveOp:
    """Custom DVE op: out = (in0 - (in0 > 1)) * s0 + in1  (one LIF step)."""
    name = "LIF_STEP_ANT_X"
    if name in _SUB_OPCODE_FOR_NAME:
        return next(op for op in OPS if op.name == name)
    body = (Src0 - (Src0 > One)) * C0 + Src1

    def _ref(in0, in1=None, s0=0.0, s1=0.0, imm2=0.0):
        return ((in0 - (in0 > 1.0).astype(np.float32)) * np.float32(s0) + in1).astype(
            np.float32
        )

    spec = Spec(body=body, reference=_ref)
    opcode = 1 + len(OPS)
    sha = {
        ver: DveOpSpec(
            name=name, opcode=opcode, uops=lower(spec, ver=ver), rd1_en=_has_src1(spec)
        ).sha(ver)
        for ver in ("v3", "v4")
    }
    op = DveOp(name, spec, subdim=False, uops_sha=sha)
    OPS.append(op)
    _SUB_OPCODE_FOR_NAME[name] = opcode
    dve_ops.CUSTOM_DVE_SPECS[name] = spec
    return op


LIF_OP = _register_lif_op()


def build_core_program(bench_iters: int = 1):
    """Build the per-core Bass program (identical on all 8 cores)."""
    nc = bacc.Bacc("TRN2", target_bir_lowering=False, debug=False)

    x = nc.dram_tensor("x", [SIZES[0], TB], FH, kind="ExternalInput").ap()
    wdram = []
    for i in range(3):
        wh = nc.dram_tensor(f"w{i}h", [SIZES[i], SIZES[i + 1]], FH,
                            kind="ExternalInput").ap()
        wl = nc.dram_tensor(f"w{i}l", [SIZES[i], SIZES[i + 1]], FH,
                            kind="ExternalInput").ap()
        wdram.append((wh, wl))
    b0p = nc.dram_tensor("b0p", [128, SIZES[1] // 128], FP, kind="ExternalInput").ap()
    b1p = nc.dram_tensor("b1p", [128, SIZES[2] // 128], FP, kind="ExternalInput").ap()
    b2p = nc.dram_tensor("b2p", [128, SIZES[3] // 128], FP, kind="ExternalInput").ap()
    # out[p, t*32 + oc*4 + b] ; output spike n = oc*128 + p
    out = nc.dram_tensor("out", [128, T * BL * (SIZES[3] // 128)], FP,
                         kind="ExternalOutput").ap()
    assert tuple(out.shape) == (128, 4096)

    bdram = [b0p, b1p, b2p]

    with tile.TileContext(nc) as tc:
        with (
            tc.tile_pool(name="xs", bufs=2) as xs_pool,        # x chunks, later s2 chunks
            tc.tile_pool(name="spk", bufs=4) as s_pool,        # s0/s1 chunk tiles
            tc.tile_pool(name="syn", bufs=3) as syn_pool,      # syn chunk tiles
            tc.tile_pool(name="w", bufs=8) as w_pool,          # weight col-group tiles
            tc.tile_pool(name="small", bufs=1) as small_pool,  # biases, zero state
            tc.tile_pool(name="mst", bufs=2) as m_pool,        # membrane ping-pong
            tc.tile_pool(name="ps", bufs=8, space="PSUM") as psum_pool,
        ):
            def _emit_body():
                # --- input chunks: xc[c][p, kc*TC*BL + tt*BL + b] (fp16, pre-scaled) ---
                rhs_tiles = []
                kc0 = SIZES[0] // 128
                xv = x.rearrange("(kc p) tb -> p kc tb", p=128)
                for c in range(NCH):
                    xt = xs_pool.tile([128, kc0 * TC * BL], FH, name=f"xc{c}", tag="xs")
                    nc.sync.dma_start(
                        xt[:].rearrange("p (kc j) -> p kc j", kc=kc0),
                        xv[:, :, c * TC * BL:(c + 1) * TC * BL],
                    )
                    rhs_tiles.append(xt)

                # --- biases and zero state (needed only by the first eviction) ---
                b_tiles = []
                for l in range(3):
                    oc_n = SIZES[l + 1] // 128
                    bt = small_pool.tile([128, oc_n], FP, name=f"bias{l}")
                    nc.sync.dma_start(bt[:], bdram[l][:])
                    b_tiles.append(bt)
                m_zero = small_pool.tile([128, 2048 // 128 * BL], FP, name="m_zero")
                nc.vector.memset(m_zero[:], 0.0)

                m_prev = [None, None, None]

                def emit_scan_step(l, c, t, syn_t, W, spike_dst):
                    tt = t - c * TC
                    m_new = m_pool.tile([128, W], FP, name=f"m{l}_{t}", tag=f"m{l}", bufs=2)
                    in0 = m_prev[l] if m_prev[l] is not None else m_zero[:, :W]
                    nc.vector._custom_dve(
                        LIF_OP, out=m_new[:], in0=in0,
                        in1=syn_t[:, tt * W:(tt + 1) * W], s0=BETA,
                    )
                    m_prev[l] = m_new[:]
                    # spike: s = (m > 1); hidden layers emit s * 2^-12 in fp16 to
                    # pair with the 2^12-scaled weight halves; the final layer
                    # emits plain 0/1 fp32 (the kernel output).
                    if l == 2:
                        nc.vector.tensor_scalar(
                            spike_dst(tt, m_new), m_new[:],
                            THRESH, None, op0=mybir.AluOpType.is_gt,
                        )
                    else:
                        nc.vector.tensor_scalar(
                            spike_dst(tt, m_new),
                            m_new[:].rearrange("p (kc b) -> p kc b", b=BL),
                            THRESH, SINV,
                            op0=mybir.AluOpType.is_gt, op1=mybir.AluOpType.mult,
                        )

                for l in range(3):
                    KC = SIZES[l] // 128
                    OC = SIZES[l + 1] // 128
                    W = OC * BL          # scan tile free width: 64 (l0/l1), 32 (l2)
                    G = OC // OCG
                    NW = TC * BL         # 256, matmul free width per chunk

                    syn_tiles = [
                        syn_pool.tile([128, TC * W], FP, name=f"syn{l}_{c}", tag="syn")
                        for c in range(NCH)
                    ]

                    # --- matmuls: groups of OCG output chunks, weights read once;
                    #     each K-chunk does two fp16 passes (hi, lo).
                    #     Layers 0/2 keep the whole layer's weights resident and
                    #     run chunk-outer so the scan of chunk 0 can start after
                    #     ~half the layer's matmuls; layer 1 (too big for that)
                    #     runs group-outer. ---
                    GW = OCG * 128     # 512, output columns per group
                    KH = KC // 2       # K-chunks per weight half-tile

                    def dma_w_group(g):
                        # two DMAs per (group, half): K split in halves so pool
                        # slots free earlier and SP-queue waits stay short.
                        # Issue in first-use order: (q0,h0), (q0,h1), (q1,h0),
                        # (q1,h1) — the kc loop consumes h0 then h1 per kc.
                        pair = [[None, None], [None, None]]
                        for q in range(2):
                            for h, wd in enumerate(wdram[l]):
                                wv = wd.rearrange("(kc p) o -> p kc o", p=128)
                                wt = w_pool.tile([128, KH * GW], FH,
                                                 name=f"w{l}g{g}h{h}q{q}",
                                                 tag="w0" if l == 0 else "w12",
                                                 bufs=8)
                                nc.sync.dma_start(
                                    wt[:].rearrange("p (kc o) -> p kc o", kc=KH),
                                    wv[:, q * KH:(q + 1) * KH, g * GW:(g + 1) * GW],
                                )
                                pair[h][q] = wt
                        return pair

                    def mm_group(g, c, pair):
                        pss = [
                            psum_pool.tile([128, NW], FP,
                                           name=f"ps{l}g{g}c{c}o{o}", tag="ps")
                            for o in range(OCG)
                        ]
                        for kc in range(KC):
                            for h in range(2):
                                wt = pair[h][kc // KH]
                                ko = (kc % KH) * GW
                                for o in range(OCG):
                                    nc.tensor.matmul(
                                        pss[o][:],
                                        wt[:, ko + o * 128:ko + (o + 1) * 128],
                                        rhs_tiles[c][:, kc * NW:(kc + 1) * NW],
                                        start=(kc == 0 and h == 0),
                                        stop=(kc == KC - 1 and h == 1),
                                    )
                        for o in range(OCG):
                            oc = g * OCG + o
                            nc.scalar.activation(
                                syn_tiles[c][:].rearrange(
                                    "p (tt w) -> p tt w", w=W
                                )[:, :, oc * BL:(oc + 1) * BL],
                                pss[o][:].rearrange("p (tt b) -> p tt b", b=BL),
                                mybir.ActivationFunctionType.Identity,
                                bias=b_tiles[l][:, oc:oc + 1], scale=1.0,
                            )

                    # chunk-outer everywhere: weights are re-read per chunk
                    # (2x weight DMA, still under the PE time) so chunk 0's
                    # scan starts after only half the layer's matmuls.
                    for c in range(NCH):
                        for g in range(G):
                            pair = dma_w_group(g)
                            mm_group(g, c, pair)

                    # --- scan + spikes ---
                    if l < 2:
                        new_rhs = []
                        for c in range(NCH):
                            st = s_pool.tile([128, OC * NW], FH,
                                             name=f"s{l}_{c}", tag="spk")
                            sv = st[:].rearrange("p (kc tb) -> p kc tb", kc=OC)

                            def spike_dst(tt, m_new, sv=sv):
                                return sv[:, :, tt * BL:(tt + 1) * BL]

                            for t in range(c * TC, (c + 1) * TC):
                                emit_scan_step(l, c, t, syn_tiles[c][:], W, spike_dst)
                            new_rhs.append(st)
                        rhs_tiles = new_rhs
                    else:
                        for c in range(NCH):
                            st = xs_pool.tile([128, TC * W], FP, name=f"s2_{c}", tag="xs")

                            def spike_dst(tt, m_new, st=st):
                                return st[:, tt * W:(tt + 1) * W]

                            for t in range(c * TC, (c + 1) * TC):
                                emit_scan_step(l, c, t, syn_tiles[c][:], W, spike_dst)
                            nc.sync.dma_start(
                                out[:, c * TC * W:(c + 1) * TC * W], st[:]
                            )


            if bench_iters > 1:
                with tc.For_i(0, bench_iters, 1):
                    _emit_body()
            else:
                _emit_body()

    nc.compile()
    return nc


_CACHE = {}


def _get_nc():
    if "nc" not in _CACHE:
        _CACHE["nc"] = build_core_program()
    return _CACHE["nc"]


def _prep_in_maps(inputs):
    x_full = np.asarray(inputs["input_spikes"], dtype=np.float32)  # [T, B, 1024]
    w = [np.asarray(inputs[f"w{i}"], dtype=np.float32) for i in range(3)]
    b = [np.asarray(inputs[f"b{i}"], dtype=np.float32) for i in range(3)]
    bp = [np.ascontiguousarray(bi.reshape(-1, 128).T) for bi in b]
    wsplit = []
    for wi in w:
        ws = np.ascontiguousarray(wi.T) * np.float32(WSCALE)   # [in, out] * 2^12
        hi = ws.astype(np.float16)
        lo = (ws - hi.astype(np.float32)).astype(np.float16)
        wsplit.append((hi, lo))
    in_maps = []
    for c in range(N_CORES):
        xs = x_full[:, c * BL:(c + 1) * BL, :]          # [T, BL, 1024]
        xp = np.ascontiguousarray(
            xs.transpose(2, 0, 1).reshape(SIZES[0], TB)  # x[n, t*BL + b]
        ).astype(np.float16) * np.float16(SINV)
        im = {"x": xp, "b0p": bp[0], "b1p": bp[1], "b2p": bp[2]}
        for i in range(3):
            im[f"w{i}h"], im[f"w{i}l"] = wsplit[i]
        in_maps.append(im)
    return in_maps


def _gather(results):
    y = np.empty((T, B, SIZES[3]), dtype=np.float32)
    for c in range(N_CORES):
        o = results[c]["out"]                           # [128, 4096]
        y[:, c * BL:(c + 1) * BL, :] = (
            o.reshape(128, T, SIZES[3] // 128, BL)      # [p, t, oc, b]
            .transpose(1, 3, 2, 0)                      # [t, b, oc, p]
            .reshape(T, BL, SIZES[3])
        )
    return y


def kernel(**inputs) -> np.ndarray:
    nc = _get_nc()
    in_maps = _prep_in_maps(inputs)
    res = run_bass_kernel_spmd(nc, in_maps, core_ids=list(range(N_CORES)))
    return _gather(res.results)


def make_timed_runner():
    """Build a reusable jitted 8-core callable (mirrors
    bass2jax.run_bass_via_pjrt's multi-core path) so repeated executions can
    be timed without re-tracing. Returns (run_from_maps, time_once)."""
    import jax
    import concourse.mybir as mybir_
    from jax.sharding import Mesh, PartitionSpec
    from jax.experimental.shard_map import shard_map
    from concourse.bass2jax import (
        _bass_exec_p,
        install_neuronx_cc_hook,
        partition_id_tensor,
    )

    install_neuronx_cc_hook()
    nc = _get_nc()
    partition_name = (
        nc.partition_id_tensor.name if nc.partition_id_tensor else None
    )

    in_names, out_names, out_avals, zero_outs = [], [], [], []
    for alloc in nc.m.functions[0].allocations:
        if not isinstance(alloc, mybir_.MemoryLocationSet):
            continue
        name = alloc.memorylocations[0].name
        if alloc.kind == "ExternalInput":
            if name != partition_name:
                in_names.append(name)
        elif alloc.kind == "ExternalOutput":
            shape = tuple(alloc.tensor_shape)
            dtype = mybir_.dt.np(alloc.dtype)
            out_names.append(name)
            out_avals.append(jax.core.ShapedArray(shape, dtype))
            zero_outs.append(np.zeros(shape, dtype))
    n_params = len(in_names)
    all_names = in_names + out_names
    if partition_name is not None:
        all_names.append(partition_name)

    def _body(*args):
        operands = list(args)
        if partition_name is not None:
            operands.append(partition_id_tensor())
        outs = _bass_exec_p.bind(
            *operands,
            out_avals=tuple(out_avals),
            in_names=tuple(all_names),
            out_names=tuple(out_names),
            lowering_input_output_aliases=(),
            sim_require_finite=True,
            sim_require_nnan=True,
            nc=nc,
        )
        return tuple(outs)

    devices = jax.devices()[:N_CORES]
    mesh = Mesh(np.asarray(devices), ("core",))
    n_outs = len(out_names)
    sharded = jax.jit(
        shard_map(
            _body, mesh=mesh,
            in_specs=(PartitionSpec("core"),) * (n_params + n_outs),
            out_specs=(PartitionSpec("core"),) * n_outs,
            check_rep=False,
        ),
        keep_unused=True,
    )

    def run_from_maps(in_maps):
        concat_in = [
            np.concatenate([np.asarray(in_maps[c][n]) for c in range(N_CORES)], axis=0)
            for n in in_names
        ]
        concat_zeros = [
            np.zeros((N_CORES * z.shape[0], *z.shape[1:]), z.dtype) for z in zero_outs
        ]
        args = [jax.device_put(a) for a in concat_in + concat_zeros]
        out = sharded(*args)
        jax.block_until_ready(out)
        results = [
            {n: np.asarray(out[i]).reshape(N_CORES, *out_avals[i].shape)[c]
             for i, n in enumerate(out_names)}
            for c in range(N_CORES)
        ]

        def time_once():
            import time as _time
            t0 = _time.perf_counter()
            o = sharded(*args)
            jax.block_until_ready(o)
            return _time.perf_counter() - t0

        return results, time_once

    return run_from_maps

